# revision 9
# baseline (speedup 1.0000x reference)
"""Causal self-attention + depthwise-conv + out-proj fused TRN2 kernel.

Model (B=4, T=2048, C=1024, H=16, D=64, conv K=4):
    qkv = x @ W_qkv.T ; causal softmax attention per head ;
    y2 = attn + causal_depthwise_conv(attn) + conv_b ; out = y2 @ W_out.T

Sharding over 8 NeuronCores: core c -> (batch b = c//2, head-group g = c%2).
Each core computes q/k/v for its 8 heads (fp32r matmuls against x[b].T),
bf16 flash-style causal attention in transposed [d, t] layout (exp softmax
without max subtraction - logits are O(1)), the depthwise conv as diagonal
matmuls along the channel partition with the residual folded into the
center tap, then a pairwise AllGather of the 512-channel activation and
half of the output projection columns.

Layout notes:
  - scores are computed transposed: S^T[k, q] = K^T.T @ Q^T so that the AV
    matmul can consume exp(S^T) directly as the moving operand.
  - causal masking is done by pre-filling the diagonal psum strip with a
    {0, -30000} staircase via an identity-stationary matmul (start=True),
    then accumulating the scores on top (start=False).
  - the AV stationary is [V_h | ones]: rows 0-63 of the psum get attn^T,
    rows 64-127 get 64 replicas of the softmax denominator, so the
    normalization is a reciprocal + lane-wise multiply.
"""

import numpy as np
import ml_dtypes

import concourse.bacc as bacc
import concourse.mybir as mybir
import concourse.tile as tile
from concourse.bass_utils import run_bass_kernel_spmd

F32R = mybir.dt.float32r
F32 = mybir.dt.float32
BF16 = mybir.dt.bfloat16

B, T, C, H, D, K = 4, 2048, 1024, 16, 64, 4
HC = H // 2  # heads per core (8)
CC = C // 2  # channels per core (512)
NEG = -30000.0
NCORES = 8
REPLICA_GROUPS = [[0, 1], [2, 3], [4, 5], [6, 7]]
NTB = T // 512  # 512-wide t blocks (4)
NTT = T // 128  # 128-wide t tiles (16)
NCT = C // 128  # 128-wide input-channel tiles (8)
NPAIR = 4  # head pairs per core

_NC_CACHE = {}


def build(debug=False):
    nc = bacc.Bacc(None, num_devices=NCORES)

    xT_d = nc.dram_tensor("xT", [C, T], F32R, kind="ExternalInput")
    wqk_d = nc.dram_tensor("wqk", [C, 1024], F32R, kind="ExternalInput")
    wv_d = nc.dram_tensor("wv", [C, CC], F32R, kind="ExternalInput")
    wout_d = nc.dram_tensor("wout", [C, CC], BF16, kind="ExternalInput")
    ident_d = nc.dram_tensor("ident", [128, 128], BF16, kind="ExternalInput")
    masks_d = nc.dram_tensor("masks", [128, 512], BF16, kind="ExternalInput")
    convdiag_d = nc.dram_tensor("convdiag", [128, NPAIR * K, 128], BF16,
                                kind="ExternalInput")
    convbias_d = nc.dram_tensor("convbias", [1, CC], BF16, kind="ExternalInput")
    outT_d = nc.dram_tensor("outT", [CC, T], BF16, kind="ExternalOutput")
    if debug:
        dbg_qT = nc.dram_tensor("dbg_qT", [128, T], F32, kind="ExternalOutput")
        dbg_kT = nc.dram_tensor("dbg_kT", [128, T], F32, kind="ExternalOutput")
        dbg_yt = nc.dram_tensor("dbg_yt", [128, T], F32, kind="ExternalOutput")
        dbg_vo = nc.dram_tensor("dbg_vo", [128, 2, 128], F32, kind="ExternalOutput")
        dbg_pt = nc.dram_tensor("dbg_pt", [128, 1024], F32, kind="ExternalOutput")
        dbg_att = nc.dram_tensor("dbg_att", [128, 512], F32, kind="ExternalOutput")

    with tile.TileContext(nc) as tc:
        with (
            tc.tile_pool(name="consts", bufs=1) as consts,
            tc.tile_pool(name="work", bufs=2) as work,
            tc.tile_pool(name="ps_st", bufs=2, space="PSUM") as ps_st,
            tc.tile_pool(name="ps_att", bufs=1, space="PSUM") as ps_att,
            tc.tile_pool(name="ps_mm", bufs=2, space="PSUM") as ps_mm,
            tc.tile_pool(name="dram", bufs=1, space="DRAM") as dram,
        ):
            # ---------- constants / big loads ----------
            xT = consts.tile([128, NCT, T], F32R, tag="xT")
            nc.sync.dma_start(xT[:], xT_d.rearrange("(n p) m -> p n m", p=128))
            ident = consts.tile([128, 128], BF16, tag="ident")
            masks = consts.tile([128, 512], BF16, tag="masks")
            nc.sync.dma_start(ident[:], ident_d[:])
            nc.sync.dma_start(masks[:], masks_d[:])
            convdiag = consts.tile([128, NPAIR * K, 128], BF16, tag="convdiag")
            nc.sync.dma_start(convdiag[:], convdiag_d[:])
            convbias = consts.tile([1, CC], BF16, tag="convbias")
            nc.sync.dma_start(convbias[:], convbias_d[:])
            onesrow = consts.tile([1, 512], BF16, tag="onesrow")
            nc.vector.memset(onesrow[:], 1.0)

            # wv shares its slot with wout (wv is dead once V is computed)
            wv = consts.tile([128, NCT, CC], F32R, tag="w2", name="wv")
            nc.sync.dma_start(wv[:], wv_d.rearrange("(n p) m -> p n m", p=128))

            # ---------- V projection into padded [V_h | ones] layout ----------
            v_ones = consts.tile([128, NTT, HC, 128], BF16, tag="v_ones")
            nc.vector.memset(v_ones[:, :, :, 0:64], 1.0)
            for tt in range(NTT):
                vps = ps_mm.tile([128, 512], F32, tag="mm", name=f"vps{tt}")
                for ct in range(NCT):
                    nc.tensor.matmul(
                        vps[:],
                        xT[:, ct, tt * 128 : tt * 128 + 128],
                        wv[:, ct, :],
                        start=(ct == 0),
                        stop=(ct == NCT - 1),
                    )
                for j in range(HC):
                    nc.vector.tensor_copy(
                        v_ones[:, tt, j, 64:128], vps[:, 64 * j : 64 * j + 64]
                    )

            # gathered-order W_out, loaded into the slot wv vacates
            wout = consts.tile([128, NCT, CC], BF16, tag="w2", name="wout")
            nc.sync.dma_start(wout[:], wout_d.rearrange("(n p) m -> p n m", p=128))

            y2g_tiles = []
            for p in range(NPAIR):
                # ---------- QK projection for head pair p ----------
                wqk = work.tile([128, NCT, 256], F32R, tag="wqk", bufs=2,
                                name=f"wqk{p}")
                nc.sync.dma_start(
                    wqk[:],
                    wqk_d[:, 256 * p : 256 * p + 256].rearrange(
                        "(n p) m -> p n m", p=128
                    ),
                )
                qT = work.tile([128, T], BF16, tag="qT", bufs=2, name=f"qT{p}")
                kT = work.tile([128, T], BF16, tag="kT", bufs=2, name=f"kT{p}")
                for fs, dst in ((0, qT), (1, kT)):
                    for tb in range(NTB):
                        ps = ps_mm.tile([128, 512], F32, tag="mm",
                                        name=f"qkps{p}_{fs}_{tb}")
                        for ct in range(NCT):
                            nc.tensor.matmul(
                                ps[:],
                                wqk[:, ct, 128 * fs : 128 * fs + 128],
                                xT[:, ct, 512 * tb : 512 * tb + 512],
                                start=(ct == 0),
                                stop=(ct == NCT - 1),
                            )
                        nc.vector.tensor_copy(
                            dst[:, 512 * tb : 512 * tb + 512], ps[:]
                        )

                if debug and p == 0:
                    for nm, src, dst in (("dq", qT, dbg_qT), ("dk", kT, dbg_kT)):
                        dtile = work.tile([128, T], F32, tag="dbg", bufs=1,
                                          name=f"dbg{nm}")
                        nc.vector.tensor_copy(dtile[:], src[:])
                        nc.sync.dma_start(dst[:], dtile[:])
                    dvo = work.tile([128, 2, 128], F32, tag="dbgv", name="dbgvo")
                    nc.vector.tensor_copy(dvo[:], v_ones[:, 0, 0:2, :])
                    nc.sync.dma_start(dbg_vo[:], dvo[:])

                # ---------- attention for the two heads of pair p ----------
                yt = work.tile([128, T], BF16, tag="yt", bufs=2, name=f"yt{p}")
                for qb in range(NTB):
                    q0 = 512 * qb
                    att = [
                        ps_att.tile([128, 512], F32, tag=f"att{h}", bufs=1,
                                    name=f"att{p}_{qb}_{h}")
                        for h in range(2)
                    ]
                    ngrp = 2 * qb + 2
                    for grp in range(ngrp):
                        for h in range(2):
                            hp = 64 * h
                            st = ps_st.tile([128, 1024], F32, tag="st", bufs=2,
                                            name=f"st{p}_{qb}_{grp}_{h}")
                            w0s = []
                            for half in range(2):
                                kt = 2 * grp + half
                                w0 = max(0, 128 * (kt - 4 * qb))
                                w0s.append(w0)
                                base = 512 * half
                                if w0 > 0 or kt >= 4 * qb:
                                    # diagonal tile: mask prefill
                                    i = kt - 4 * qb
                                    nc.tensor.matmul(
                                        st[:, base + w0 : base + w0 + 128],
                                        ident[:],
                                        masks[:, 128 * i : 128 * i + 128],
                                        start=True,
                                        stop=False,
                                    )
                                    sc_start = False
                                else:
                                    sc_start = True
                                nc.tensor.matmul(
                                    st[:, base + w0 : base + 512],
                                    kT[hp : hp + 64, 128 * kt : 128 * kt + 128],
                                    qT[hp : hp + 64, q0 + w0 : q0 + 512],
                                    start=sc_start,
                                    stop=True,
                                )
                            pt = work.tile([128, 1024], BF16, tag="pt", bufs=4,
                                           name=f"pt{p}_{qb}_{grp}_{h}")
                            nc.scalar.activation(
                                out=pt[:, w0s[0] : 1024],
                                in_=st[:, w0s[0] : 1024],
                                func=mybir.ActivationFunctionType.Exp,
                                scale=0.125,
                            )
                            if debug and p == 0 and qb == 0 and grp == 0 and h == 0:
                                dpt = work.tile([128, 1024], F32, tag="dbgpt",
                                                name="dbgpt")
                                nc.vector.tensor_copy(dpt[:], pt[:])
                                nc.sync.dma_start(dbg_pt[:], dpt[:])
                            for half in range(2):
                                kt = 2 * grp + half
                                w0 = w0s[half]
                                base = 512 * half
                                nc.tensor.matmul(
                                    att[h][:, w0:512],
                                    v_ones[:, kt, 2 * p + h, :],
                                    pt[:, base + w0 : base + 512],
                                    start=(kt == 0),
                                    stop=(kt == 4 * qb + 3),
                                )
                    if debug and p == 0 and qb == 0:
                        datt = work.tile([128, 512], F32, tag="dbgatt",
                                         name="dbgatt")
                        nc.vector.tensor_copy(datt[:], att[0][:])
                        nc.sync.dma_start(dbg_att[:], datt[:])
                    for h in range(2):
                        rec = work.tile([64, 512], F32, tag="rec", bufs=2,
                                        name=f"rec{p}_{qb}_{h}")
                        nc.vector.reciprocal_approx_fast(rec[:], att[h][0:64, :])
                        nc.vector.tensor_mul(
                            out=yt[64 * h : 64 * h + 64, q0 : q0 + 512],
                            in0=att[h][64:128, :],
                            in1=rec[:],
                        )

                if debug and p == 0:
                    dyt = work.tile([128, T], F32, tag="dbg", bufs=1,
                                    name="dbgyt")
                    nc.vector.tensor_copy(dyt[:], yt[:])
                    nc.sync.dma_start(dbg_yt[:], dyt[:])

                # ---------- depthwise causal conv + residual + bias ----------
                y2loc = dram.tile([128, T], BF16, tag=f"y2loc{p}",
                                  name=f"y2loc{p}")
                for tb in range(NTB):
                    t0 = 512 * tb
                    cps = ps_mm.tile([128, 512], F32, tag="mm",
                                     name=f"cps{p}_{tb}")
                    for lag in range(4):
                        j = 3 - lag  # tap index; lag 0 tap has +1 residual
                        lo = max(0, lag - t0)
                        nc.tensor.matmul(
                            cps[:, lo:512],
                            convdiag[:, K * p + j, :],
                            yt[:, t0 + lo - lag : t0 + 512 - lag],
                            start=(lag == 0),
                            stop=False,
                        )
                    nc.tensor.matmul(
                        cps[:],
                        convbias[:, 128 * p : 128 * p + 128],
                        onesrow[:],
                        start=False,
                        stop=True,
                    )
                    y2sb = work.tile([128, 512], BF16, tag="y2sb", bufs=4,
                                     name=f"y2sb{p}_{tb}")
                    nc.scalar.activation(
                        out=y2sb[:],
                        in_=cps[:],
                        func=mybir.ActivationFunctionType.Copy,
                    )
                    nc.sync.dma_start(y2loc[:, t0 : t0 + 512], y2sb[:])

                # ---------- pairwise AllGather of this 128-channel slab ----------
                y2g = dram.tile([256, T], BF16, tag=f"y2g{p}", name=f"y2g{p}")
                nc.gpsimd.collective_compute(
                    "AllGather",
                    mybir.AluOpType.bypass,
                    replica_groups=REPLICA_GROUPS,
                    ins=[y2loc.opt()],
                    outs=[y2g.opt()],
                )
                y2g_tiles.append(y2g)

            # ---------- output projection (this core's 512 columns) ----------
            for tb in range(NTB):
                t0 = 512 * tb
                for ot in range(4):
                    ops_ = ps_mm.tile([128, 512], F32, tag="mm",
                                      name=f"ops{tb}_{ot}")
                    for gs in range(8):
                        p, parity = gs // 2, gs % 2
                        ysb = work.tile([128, 512], BF16, tag="ysb", bufs=4,
                                        name=f"ysb{tb}_{ot}_{gs}")
                        nc.sync.dma_start(
                            ysb[:],
                            y2g_tiles[p][128 * parity : 128 * parity + 128,
                                         t0 : t0 + 512],
                        )
                        nc.tensor.matmul(
                            ops_[:],
                            wout[:, gs, 128 * ot : 128 * ot + 128],
                            ysb[:],
                            start=(gs == 0),
                            stop=(gs == 7),
                        )
                    osb = work.tile([128, 512], BF16, tag="osb", bufs=4,
                                    name=f"osb{tb}_{ot}")
                    nc.vector.tensor_copy(osb[:], ops_[:])
                    nc.sync.dma_start(
                        outT_d[128 * ot : 128 * ot + 128, t0 : t0 + 512], osb[:]
                    )

    nc.compile()
    return nc


def _make_masks():
    kp = np.arange(128)[:, None]
    col = np.arange(128)[None, :]
    masks = np.zeros((128, 512), np.float32)
    for i in range(4):
        masks[:, 128 * i : 128 * i + 128] = np.where(kp > col, NEG, 0.0)
    return masks.astype(ml_dtypes.bfloat16)


def prepare_in_maps(x, W_qkv, W_out, conv_w, conv_b):
    x = np.asarray(x, np.float32)
    W_qkv = np.asarray(W_qkv, np.float32)
    W_out = np.asarray(W_out, np.float32)
    conv_w = np.asarray(conv_w, np.float32).reshape(C, K)
    conv_b = np.asarray(conv_b, np.float32)

    ident = np.eye(128, dtype=np.float32).astype(ml_dtypes.bfloat16)
    masks = _make_masks()

    # gathered channel order: row r of y2g stack -> global channel
    perm = np.empty(C, np.int64)
    for r in range(C):
        p, parity, within = r // 256, (r % 256) // 128, r % 128
        perm[r] = 512 * parity + 128 * p + within

    in_maps = []
    for core in range(NCORES):
        b, g = core // 2, core % 2
        xT = np.ascontiguousarray(x[b].T)  # [C, T]
        # wqk: cols [256p:256p+128] = q rows of pair p (.T), then k rows
        wqk = np.empty((C, 1024), np.float32)
        for p in range(NPAIR):
            r0 = 64 * (8 * g + 2 * p)
            wqk[:, 256 * p : 256 * p + 128] = W_qkv[r0 : r0 + 128, :].T
            wqk[:, 256 * p + 128 : 256 * p + 256] = W_qkv[
                1024 + r0 : 1024 + r0 + 128, :
            ].T
        wv = np.ascontiguousarray(W_qkv[2048 + CC * g : 2048 + CC * g + CC, :].T)
        # W_out columns for this core's output slice, rows in gathered order
        wout = np.ascontiguousarray(
            W_out[CC * g : CC * g + CC, :].T[perm, :]
        ).astype(ml_dtypes.bfloat16)
        # conv diag matrices for this core's 4 channel tiles x 4 taps
        convdiag = np.zeros((128, NPAIR * K, 128), np.float32)
        idx = np.arange(128)
        for p in range(NPAIR):
            for j in range(K):
                w = conv_w[CC * g + 128 * p : CC * g + 128 * p + 128, j]
                if j == K - 1:
                    w = w + 1.0  # residual folded into the lag-0 tap
                convdiag[idx, K * p + j, idx] = w
        convbias = conv_b[CC * g : CC * g + CC].reshape(1, CC)
        in_maps.append(
            {
                "xT": xT,
                "wqk": wqk,
                "wv": wv,
                "wout": wout,
                "ident": ident,
                "masks": masks,
                "convdiag": convdiag.astype(ml_dtypes.bfloat16),
                "convbias": convbias.astype(ml_dtypes.bfloat16),
            }
        )
    return in_maps


def assemble_output(results):
    out = np.empty((B, T, C), np.float32)
    for core in range(NCORES):
        b, g = core // 2, core % 2
        outT = np.asarray(results[core]["outT"], np.float32)  # [CC, T]
        out[b, :, CC * g : CC * g + CC] = outT.T
    return out


def kernel(x, W_qkv, W_out, conv_w, conv_b):
    if "nc" not in _NC_CACHE:
        _NC_CACHE["nc"] = build()
    nc = _NC_CACHE["nc"]
    in_maps = prepare_in_maps(x, W_qkv, W_out, conv_w, conv_b)
    res = run_bass_kernel_spmd(nc, in_maps, list(range(NCORES)))
    return assemble_output(res.results)
